# revision 28
# baseline (speedup 1.0000x reference)
"""Trainium2 Bass kernel for EnhancedLocalAttentionWithGQA (differential
windowed attention, B=2 L=4096 E=1024 H=16 G=2 W=256 D=64).

Key structural facts exploited:
  - The reference concatenates nw=31 overlapping windows along the sequence
    and trims to L=4096 = 16*W, so ONLY windows 0..15 contribute, and only
    input positions 0..2175 are used as queries/keys/values.
  - Output row p comes from window n = p//256, in-window query j = p%256,
    i.e. input position n*128 + j.

Sharding: 8 cores, core c owns windows (2c, 2c+1) -> output rows
[512c, 512c+512) for both batches. Each core needs x rows [256c, 256c+384).

On-device dataflow (per core, SPMD — all core differences come via inputs):
  - q^T / k^T computed directly in [head-dim, seq] layout (lhsT = weight
    tile, rhs = x^T tile). Host pre-permutes W columns so each head block
    is [evens | odds], making RoPE 2 full-tile muls + 4 partition-offset
    add/subs. Branch-2 weights are block-swapped so the two differential
    branches occupy complementary 64-partition halves (concurrent K=64
    score matmuls via PE row groups).
  - v computed TRANSPOSED (v^T = Wv^T x^T, N=384 full f32r rate), then
    PE-transposed back to [seq, kv] and cast to bf16 for the PV matmuls.
  - Scores computed transposed: S^T[k, q] (lhsT = k^T, rhs = q^T), exp on
    ACT without max-subtraction (scores are small) -> bf16, then PV as
    out[q, 65] = E_tile.T @ [v | ones] in bf16 (1 cyc/row instead of the
    f32r 4 cyc/row small-N penalty). The 65th (ones) column gives the TRUE
    softmax denominator for both branches; lambda is folded into the
    branch-2 reciprocal on DVE in f32.
  - Normalize+combine: one ACT copy (scale=lam/den2) + one DVE
    scalar_tensor_tensor -> a = num1/den1 - num2*(lam/den2), written bf16.
  - a transposed back via bf16 PE transpose (head pair packed into one PSUM
    tile), out-projection in bf16 accumulated over 8 K-tiles; bias added
    via DVE tensor_add from a broadcast bias tile (no PE bias matmuls),
    Y DMA'd from SBUF f32.
"""

import os
import sys

sys.path.insert(0, "/opt/trn_rl_repo")
os.environ.setdefault("MYCRO_LOCAL_CACHE", "1")

import numpy as np

B, L, E, H, G, W, D = 2, 4096, 1024, 16, 2, 256, 64
NCORES = 8
SEQ = 384          # x rows per core
NW = 2             # windows per core
QROWS = 512        # output rows per core per batch
KV = E // (H // G)  # 128
LAMBDA_INIT = 0.8

F32 = None  # set lazily (mybir import deferred so host prep works standalone)


# ----------------------------------------------------------------- host prep

def _head_perm():
    """Column permutation applied to Wq1/Wk1: per 64-block -> [evens|odds]."""
    p = []
    for blk in range(0, E, D):
        p += [blk + 2 * j for j in range(D // 2)]
        p += [blk + 2 * j + 1 for j in range(D // 2)]
    return np.array(p, dtype=np.int64)


def _q2_perm():
    """q2: like _head_perm but heads swapped within each 128-col M-tile."""
    base = _head_perm()
    p = np.empty_like(base)
    for m in range(E // 128):
        p[m * 128: m * 128 + 64] = base[m * 128 + 64: m * 128 + 128]
        p[m * 128 + 64: m * 128 + 128] = base[m * 128: m * 128 + 64]
    return p


def _k_perm(swap):
    """kv columns (128 = 2 groups x 64): per group block [evens|odds];
    swap=True puts group1 first (branch-2 layout)."""
    p = []
    groups = (1, 0) if swap else (0, 1)
    for g in groups:
        blk = g * D
        p += [blk + 2 * j for j in range(D // 2)]
        p += [blk + 2 * j + 1 for j in range(D // 2)]
    return np.array(p, dtype=np.int64)


def _tile_w(w, kdim, mdim):
    """(kdim*128, mdim*TS) -> (kdim, mdim, 128, TS) contiguous tiles."""
    ts = w.shape[1] // mdim
    return np.ascontiguousarray(
        w.reshape(kdim, 128, mdim, ts).transpose(0, 2, 1, 3))


def _trig_tables(core):
    pos = (256 * core + np.arange(SEQ, dtype=np.float64))  # global positions
    div = np.exp(np.arange(0, D, 2, dtype=np.float64) * (-np.log(10000.0) / D))
    ang = pos[None, :] * div[:, None]          # (32, SEQ)
    c32 = np.cos(ang).astype(np.float32)
    s32 = np.sin(ang).astype(np.float32)
    tc = np.tile(c32, (4, 1))                   # (128, SEQ)
    # sign-folded sin: rows [0:32]=+sin (qe*sin for the odd half),
    # [32:64]=-sin (-qo*sin for the even half), repeating per 64-block.
    tsn = np.tile(np.concatenate([s32, -s32], axis=0), (2, 1))
    return np.ascontiguousarray(tc), np.ascontiguousarray(tsn)


def _p32():
    """[128,128] permutation: swaps 32-halves within each 64-block.
    Used as matmul lhsT: out = P.T @ u with P[k, m] = 1 iff k = swap(m)."""
    p = np.zeros((128, 128), np.float32)
    for m in range(128):
        k = m + 32 if (m % 64) < 32 else m - 32
        p[k, m] = 1.0
    return p


# ------------------------------------------------------------ device program

_PROGRAM_CACHE = {}


def _build_program():
    import concourse.bass as bass
    import concourse.mybir as mybir
    import concourse.tile as tile
    from concourse.masks import make_identity
    from concourse.tile_rust import add_dep_helper

    def order_group(insts):
        """PE-order a bank-packed accumulation group: first (start=True)
        before everything, last (stop=True) after everything. sync=False —
        same-engine ordering only."""
        first, last = insts[0], insts[-1]
        for i in insts[1:]:
            add_dep_helper(i.ins, first.ins, sync=False,
                           reason="psum group start first")
        for i in insts[:-1]:
            add_dep_helper(last.ins, i.ins, sync=False,
                           reason="psum group stop last")

    f32 = mybir.dt.float32
    f32r = mybir.dt.float32r
    bf16 = mybir.dt.bfloat16
    ALU = mybir.AluOpType
    ACTF = mybir.ActivationFunctionType

    nc = bass.Bass()

    xt_d = nc.dram_tensor("xt", [B, 8, 128, SEQ], bf16, kind="ExternalInput")
    # wq pre-chunked on host: [chunk=(mat,mt//2), 128p, (mt%2, kt, 128)]
    wq_d = nc.dram_tensor("wq", [8, 128, 2048], bf16, kind="ExternalInput")
    wk_d = nc.dram_tensor("wk", [2, 8, 128, 128], bf16, kind="ExternalInput")
    wv_d = nc.dram_tensor("wv", [8, 128, 128], bf16, kind="ExternalInput")
    wo_d = nc.dram_tensor("wo", [8, 2, 128, 512], bf16, kind="ExternalInput")
    tc_d = nc.dram_tensor("tct", [128, SEQ], f32, kind="ExternalInput")
    ts_d = nc.dram_tensor("tst", [128, SEQ], f32, kind="ExternalInput")
    lam_d = nc.dram_tensor("lamv", [128, 2], f32, kind="ExternalInput")
    bout_d = nc.dram_tensor("boutv", [1, E], f32r, kind="ExternalInput")
    p32_d = nc.dram_tensor("p32", [128, 128], bf16, kind="ExternalInput")
    ones_d = nc.dram_tensor("onesv", [1, 128], f32r, kind="ExternalInput")
    y_d = nc.dram_tensor("y", [B, QROWS, E], f32, kind="ExternalOutput")

    def split_matmul_waits():
        """This walrus build allows only ONE sync-wait per engine
        instruction; peel extra waits onto engine-matched no-ops placed
        just before the instruction."""
        for bb in nc.m.functions[0].blocks:
            il = bb.instructions
            new_list = []
            changed = False
            for i in il:
                si = getattr(i, "sync_info", None)
                if si is not None and len(si.on_wait) > 1:
                    waits = list(si.on_wait)
                    for j, w in enumerate(waits[1:]):
                        nop = mybir.InstNoOp(
                            name=f"{i.name}-wnop{j}", engine=i.engine, ins=[],
                            outs=[],
                            sync_info=mybir.SyncInfo(on_wait=[w],
                                                     on_update=[]))
                        nc.inst_map[nop.name] = nop
                        new_list.append(nop)
                    i.sync_info = mybir.SyncInfo(
                        on_wait=[waits[0]], on_update=list(si.on_update))
                    changed = True
                new_list.append(i)
            if changed:
                il[:] = new_list

    with tile.TileContext(nc) as tc:
        with tc.tile_pool(name="const", bufs=1) as constp, \
             tc.tile_pool(name="xt", bufs=1) as xtp, \
             tc.tile_pool(name="rot", bufs=1) as rotp, \
             tc.tile_pool(name="wres", bufs=1) as wresp, \
             tc.tile_pool(name="ru", bufs=3) as rup, \
             tc.tile_pool(name="vext", bufs=1) as vxp, \
             tc.tile_pool(name="att", bufs=4) as attp, \
             tc.tile_pool(name="small", bufs=4) as smp, \
             tc.tile_pool(name="atile", bufs=3) as atp, \
             tc.tile_pool(name="psA", bufs=2, space="PSUM") as psA, \
             tc.tile_pool(name="psSC", bufs=2, space="PSUM") as psSC, \
             tc.tile_pool(name="psPV", bufs=3, space="PSUM") as psPV, \
             tc.tile_pool(name="psTR", bufs=1, space="PSUM") as psTR:

            # x^T + weights: one HW queue (sync), strict consumption order
            xts = {}
            xstrips = []
            for b in range(B):
                strip = xtp.tile([128, 8 * SEQ], bf16, tag=f"xt{b}")
                xstrips.append(strip)
                for kt in range(8):
                    xts[b, kt] = strip[:, kt * SEQ:(kt + 1) * SEQ]

            def dma_x(b):
                nc.sync.dma_start(
                    out=xstrips[b].rearrange("p (k s) -> p k s", k=8),
                    in_=xt_d[b, :, :, :].rearrange("k p s -> p k s"))

            wqr = wresp.tile([128, 16384], bf16, tag="wqr")

            def dma_wq(chunk):
                nc.sync.dma_start(
                    out=wqr[:, chunk * 2048:(chunk + 1) * 2048],
                    in_=wq_d[chunk, :, :])

            def wq_sb(mat, mt, kt):
                off = (mat * 4 + mt // 2) * 2048 + (mt % 2) * 1024 + kt * 128
                return wqr[:, off:off + 128]

            dma_x(0)
            dma_wq(0)
            dma_wq(1)
            tc_sb = constp.tile([128, SEQ], f32, tag="tcs")
            ts_sb = constp.tile([128, SEQ], f32, tag="tss")
            p32_sb = constp.tile([128, 128], bf16, tag="p32s")
            nc.sync.dma_start(out=tc_sb, in_=tc_d[:, :])
            nc.sync.dma_start(out=ts_sb, in_=ts_d[:, :])
            nc.sync.dma_start(out=p32_sb, in_=p32_d[:, :])
            for chunk in range(2, 8):
                dma_wq(chunk)
            dma_x(1)

            wkr = wresp.tile([128, 2048], bf16, tag="wkr")
            nc.sync.dma_start(
                out=wkr.rearrange("p (m k s) -> p m k s", m=2, k=8),
                in_=wk_d[:, :, :, :].rearrange("m k p s -> p m k s"))
            wvr = wresp.tile([128, 1024], bf16, tag="wvr")
            nc.sync.dma_start(
                out=wvr.rearrange("p (k s) -> p k s", k=8),
                in_=wv_d[:, :, :].rearrange("k p s -> p k s"))
            lam_sb = constp.tile([128, 2], f32, tag="lams")
            bout_sb = constp.tile([1, E], f32r, tag="bouts")
            ones1 = constp.tile([1, 128], f32r, tag="ones1")
            nc.sync.dma_start(out=lam_sb, in_=lam_d[:, :])
            nc.sync.dma_start(out=bout_sb, in_=bout_d[:, :])
            nc.sync.dma_start(out=ones1, in_=ones_d[:, :])
            wor = wresp.tile([128, 8192], bf16, tag="wor")
            nc.sync.dma_start(
                out=wor.rearrange("p (k n s) -> p k n s", k=8, n=2),
                in_=wo_d[:, :, :, :].rearrange("k n p s -> p k n s"))
            wo_sb = {}
            for kt in range(8):
                for nh in range(2):
                    off = kt * 1024 + nh * 512
                    wo_sb[kt, nh] = wor[:, off:off + 512]

            identf = constp.tile([128, 128], f32, tag="identf")
            make_identity(nc, identf)
            identb = constp.tile([128, 128], bf16, tag="identb")
            nc.vector.tensor_copy(identb, identf)

            def rope(psum_in, rot_out):
                # rot = psum*TC + P32 @ (psum*TS_signed)
                # (the PE matmul does the cross-partition 32-half swap that
                # DVE cannot: walrus requires same start partition on all
                # InstTensorTensor operands)
                t = rup.tile([128, SEQ], f32, tag="ropet")
                u = rup.tile([128, SEQ], bf16, tag="ropeu")
                nc.vector.tensor_mul(t, psum_in, tc_sb)
                nc.vector.tensor_mul(u, psum_in, ts_sb)
                usw = psSC.tile([128, 512], f32, tag="sc")
                nc.tensor.matmul(usw[:, 0:SEQ], p32_sb,
                                 u, start=True, stop=True)
                nc.vector.tensor_add(rot_out, t, usw[:, 0:SEQ])

            # ---- q projections + rope ----
            qrot = {}
            for b in range(B):
                for mat in range(2):
                    for mt in range(8):
                        ps = psA.tile([128, 512], f32, tag="proj")
                        qp = ps[:, 0:SEQ]
                        for kt in range(8):
                            nc.tensor.matmul(
                                qp, wq_sb(mat, mt, kt), xts[b, kt],
                                start=(kt == 0), stop=(kt == 7))
                        rot = rotp.tile([128, SEQ], bf16, tag=f"q{mat}_{b}_{mt}")
                        rope(qp, rot)
                        qrot[mat, b, mt] = rot

            # ---- v projections (transposed, full-rate) + v_ext tiles ----
            vext = {}
            for b in range(B):
                # v^T [kv=128, SEQ] at full bf16 rate (N=384)
                ps = psA.tile([128, 512], f32, tag="proj")
                vtp = ps[:, 0:SEQ]
                for kt in range(8):
                    nc.tensor.matmul(
                        vtp, wvr[:, kt * 128:(kt + 1) * 128],
                        xts[b, kt],
                        start=(kt == 0), stop=(kt == 7))
                vt_sb = rup.tile([128, SEQ], bf16, tag="vtsb")
                nc.vector.tensor_copy(vt_sb, vtp)
                # PE-transpose each 128-seq chunk back to [seq, kv]
                for st in range(3):
                    vtr = psTR.tile([128, 256], bf16, tag="trp")
                    nc.tensor.matmul(vtr[:, 0:128],
                                     vt_sb[:, st * 128:(st + 1) * 128],
                                     identb, is_transpose=True)
                    for g in range(2):
                        ve = vxp.tile([128, 65], bf16,
                                      tag=f"ve_{b}_{st}_{g}")
                        nc.vector.tensor_copy(ve[:, 0:64],
                                              vtr[:, g * 64:(g + 1) * 64])
                        nc.gpsimd.tensor_copy(ve[:, 64:65],
                                              lam_sb[:, 0:1])
                        vext[b, st, g] = ve

            # ---- k projections + rope ----
            krot = {}
            for mat in range(2):
                for b in range(B):
                    ps = psA.tile([128, 512], f32, tag="proj")
                    kp = ps[:, 0:SEQ]
                    for kt in range(8):
                        nc.tensor.matmul(
                            kp, wkr[:, mat * 1024 + kt * 128:
                                    mat * 1024 + kt * 128 + 128],
                            xts[b, kt],
                            start=(kt == 0), stop=(kt == 7))
                    rot = rotp.tile([128, SEQ], bf16, tag=f"k{mat}_{b}")
                    rope(kp, rot)
                    krot[mat, b] = rot

            # broadcast bias tile [128, 1024] via two K=1 matmuls (one-time)
            bias_sb = constp.tile([128, E], f32, tag="biasbc")
            for nh in range(2):
                bps = psA.tile([128, 512], f32, tag="proj")
                nc.tensor.matmul(bps, ones1,
                                 bout_sb[:, nh * 512:(nh + 1) * 512],
                                 start=True, stop=True)
                nc.vector.tensor_copy(bias_sb[:, nh * 512:(nh + 1) * 512],
                                      bps)

            # ---- attention + output projection ----
            for b in range(B):
                for w in range(NW):
                    at_sb = {}
                    for h in range(H):
                        g = h & 1
                        mt = h >> 1
                        base1 = 64 * g          # branch-1 partition base
                        base2 = 64 - base1      # branch-2 partition base
                        e_sb = []
                        for br, qb in ((0, base1), (1, base2)):
                            st_ps = psSC.tile([128, 512], f32, tag="sc")
                            sc_mms = []
                            for kts in range(2):
                                sc_mms.append(nc.tensor.matmul(
                                    st_ps[:, kts * 256:(kts + 1) * 256],
                                    krot[br, b][qb:qb + 64,
                                                w * 128 + kts * 128:
                                                w * 128 + kts * 128 + 128]
                                    ,
                                    qrot[br, b, mt][qb:qb + 64,
                                                    w * 128:w * 128 + 256]
                                    ,
                                    start=(kts == 0), stop=(kts == 1)))
                            order_group(sc_mms)
                            e = attp.tile([128, 512], bf16, tag=f"e{br}")
                            nc.scalar.activation(e, st_ps, ACTF.Exp,
                                                 scale=0.125)
                            e_sb.append(e)

                        pv = psPV.tile([128, 260], f32, tag="pv")
                        pv_mms = []
                        first = True
                        for kts in range(2):
                            for br in range(2):
                                for qt in range(2):
                                    col = (br * 2 + qt) * 65
                                    pv_mms.append(nc.tensor.matmul(
                                        pv[:, col:col + 65],
                                        e_sb[br][:, kts * 256 + qt * 128:
                                                 kts * 256 + qt * 128 + 128]
                                        ,
                                        vext[b, w + kts, g],
                                        start=first,
                                        stop=(kts == 1 and br == 1 and qt == 1)))
                                    first = False
                        order_group(pv_mms)

                        r_sb = smp.tile([128, 6], f32, tag="recip")
                        nc.vector.reciprocal(r_sb[:, 0:4], pv[:, 64:260:65])
                        # fold lambda into the branch-2 reciprocals (f32)
                        nc.vector.tensor_scalar_mul(
                            r_sb[:, 4:6], r_sb[:, 2:4], lam_sb[:, 1:2])

                        if g == 0:
                            # [128 q, 128] pair tile: h-even dims in cols
                            # 0:64, h-odd in 64:128; transposed in one shot.
                            pair_sb = [smp.tile([128, 128], bf16,
                                                tag=f"pair{qt}",
                                                name=f"pair{qt}")
                                       for qt in range(2)]
                        for qt in range(2):
                            t2 = smp.tile([128, 64], f32, tag="t2")
                            nc.scalar.activation(
                                t2, pv[:, 130 + qt * 65:130 + qt * 65 + 64],
                                ACTF.Copy, scale=r_sb[:, 4 + qt:5 + qt])
                            nc.vector.scalar_tensor_tensor(
                                out=pair_sb[qt][:, g * 64:(g + 1) * 64],
                                in0=pv[:, qt * 65:qt * 65 + 64],
                                scalar=r_sb[:, qt:qt + 1], in1=t2,
                                op0=ALU.mult, op1=ALU.subtract)
                        if g == 1:
                            tr_ps = psTR.tile([128, 256], bf16, tag="trp")
                            for qt in range(2):
                                nc.tensor.transpose(
                                    tr_ps[:, qt * 128:(qt + 1) * 128],
                                    pair_sb[qt], identb)
                                at = atp.tile([128, 128], bf16,
                                              tag=f"at{mt}_{qt}")
                                nc.vector.tensor_copy(
                                    at, tr_ps[:, qt * 128:(qt + 1) * 128])
                                at_sb[mt, qt] = at

                    for qt in range(2):
                        for nh in range(2):
                            y_ps = psA.tile([128, 512], f32, tag="proj")
                            y_mms = []
                            for kt in range(8):
                                y_mms.append(nc.tensor.matmul(
                                    y_ps, at_sb[kt, qt],
                                    wo_sb[kt, nh],
                                    start=(kt == 0), stop=(kt == 7)))
                            order_group(y_mms)
                            y_sb = smp.tile([128, 512], f32, tag="ysb")
                            nc.vector.tensor_add(
                                y_sb, y_ps,
                                bias_sb[:, nh * 512:(nh + 1) * 512])
                            nc.sync.dma_start(
                                out=y_d[b, (w * 2 + qt) * 128:
                                        (w * 2 + qt) * 128 + 128,
                                        nh * 512:(nh + 1) * 512],
                                in_=y_sb)
    split_matmul_waits()
    return nc


def get_program():
    if "nc" not in _PROGRAM_CACHE:
        _PROGRAM_CACHE["nc"] = _build_program()
    return _PROGRAM_CACHE["nc"]


# ------------------------------------------------------------------ host API

def make_in_maps(x, Wq1, Wq2, Wk1, Wk2, Wv, Wout, bout, lq1, lk1, lq2, lk2):
    import ml_dtypes
    bf16 = ml_dtypes.bfloat16

    x = np.asarray(x, dtype=np.float32)
    lam = float(np.clip(
        np.exp(np.asarray(lq1, np.float64) @ np.asarray(lk1, np.float64))
        - np.exp(np.asarray(lq2, np.float64) @ np.asarray(lk2, np.float64))
        + LAMBDA_INIT, 0.1, 0.9))

    qp1, qp2 = _head_perm(), _q2_perm()
    kp1, kp2 = _k_perm(False), _k_perm(True)

    wq_t = np.stack([
        _tile_w(np.asarray(Wq1, np.float32)[:, qp1], 8, 8),
        _tile_w(np.asarray(Wq2, np.float32)[:, qp2], 8, 8),
    ])  # (mat, kt, mt, 128, 128)
    # chunk layout for single contiguous DMAs:
    # (mat, mt//2, p, mt%2, kt, s) -> (8, 128, 2048)
    wq = np.ascontiguousarray(
        wq_t.reshape(2, 8, 4, 2, 128, 128)
        .transpose(0, 2, 4, 3, 1, 5).reshape(8, 128, 2048)).astype(bf16)
    wk = np.stack([
        _tile_w(np.asarray(Wk1, np.float32)[:, kp1], 8, 1)[:, 0],
        _tile_w(np.asarray(Wk2, np.float32)[:, kp2], 8, 1)[:, 0],
    ]).astype(bf16)  # (2, 8, 128, 128)
    wv = _tile_w(np.asarray(Wv, np.float32), 8, 1)[:, 0].astype(bf16)
    wo = _tile_w(np.asarray(Wout, np.float32), 8, 2).astype(bf16)
    boutv = np.asarray(bout, np.float32).reshape(1, E)

    lamv = np.zeros((128, 2), np.float32)
    lamv[:, 0] = 1.0     # exact ones column for the shared denominator
    lamv[:, 1] = lam     # lambda, applied in f32 on DVE

    # x^T, tiled: (B, 8, 128, SEQ) per core
    xT = np.ascontiguousarray(x.transpose(0, 2, 1))  # (B, E, L)

    in_maps = []
    for c in range(NCORES):
        s0 = 256 * c
        xt = np.ascontiguousarray(
            xT[:, :, s0:s0 + SEQ].reshape(B, 8, 128, SEQ)).astype(bf16)
        tct, tst = _trig_tables(c)
        in_maps.append({
            "xt": xt, "wq": wq, "wk": wk, "wv": wv, "wo": wo,
            "tct": tct, "tst": tst, "lamv": lamv, "boutv": boutv,
            "p32": _p32().astype(bf16),
            "onesv": np.ones((1, 128), np.float32),
        })
    return in_maps


def kernel(**inputs) -> np.ndarray:
    from concourse.bass_utils import run_bass_kernel_spmd

    in_maps = make_in_maps(**inputs)
    nc = get_program()
    res = run_bass_kernel_spmd(nc, in_maps, core_ids=list(range(NCORES)))
    out = np.empty((B, L, E), dtype=np.float32)
    for c in range(NCORES):
        out[:, 512 * c:512 * (c + 1), :] = res.results[c]["y"]
    return out


# revision 30
# speedup vs baseline: 1.0879x; 1.0879x over previous
"""Trainium2 Bass kernel for EnhancedLocalAttentionWithGQA (differential
windowed attention, B=2 L=4096 E=1024 H=16 G=2 W=256 D=64).

Key structural facts exploited:
  - The reference concatenates nw=31 overlapping windows along the sequence
    and trims to L=4096 = 16*W, so ONLY windows 0..15 contribute, and only
    input positions 0..2175 are used as queries/keys/values.
  - Output row p comes from window n = p//256, in-window query j = p%256,
    i.e. input position n*128 + j.

Sharding: 8 cores, core c owns windows (2c, 2c+1) -> output rows
[512c, 512c+512) for both batches. Each core needs x rows [256c, 256c+384).

On-device dataflow (per core, SPMD — all core differences come via inputs):
  - q^T / k^T computed directly in [head-dim, seq] layout (lhsT = weight
    tile, rhs = x^T tile). Host pre-permutes W columns so each head block
    is [evens | odds], making RoPE 2 full-tile muls + 4 partition-offset
    add/subs. Branch-2 weights are block-swapped so the two differential
    branches occupy complementary 64-partition halves (concurrent K=64
    score matmuls via PE row groups).
  - v computed TRANSPOSED (v^T = Wv^T x^T, N=384 full f32r rate), then
    PE-transposed back to [seq, kv] and cast to bf16 for the PV matmuls.
  - Scores computed transposed: S^T[k, q] (lhsT = k^T, rhs = q^T), exp on
    ACT without max-subtraction (scores are small) -> bf16, then PV as
    out[q, 65] = E_tile.T @ [v | ones] in bf16 (1 cyc/row instead of the
    f32r 4 cyc/row small-N penalty). The 65th (ones) column gives the TRUE
    softmax denominator for both branches; lambda is folded into the
    branch-2 reciprocal on DVE in f32.
  - Normalize+combine: one ACT copy (scale=lam/den2) + one DVE
    scalar_tensor_tensor -> a = num1/den1 - num2*(lam/den2), written bf16.
  - a transposed back via bf16 PE transpose (head pair packed into one PSUM
    tile), out-projection in bf16 accumulated over 8 K-tiles; bias added
    via DVE tensor_add from a broadcast bias tile (no PE bias matmuls),
    Y DMA'd from SBUF f32.
"""

import os
import sys

sys.path.insert(0, "/opt/trn_rl_repo")
os.environ.setdefault("MYCRO_LOCAL_CACHE", "1")

import numpy as np

B, L, E, H, G, W, D = 2, 4096, 1024, 16, 2, 256, 64
NCORES = 8
SEQ = 384          # x rows per core
NW = 2             # windows per core
QROWS = 512        # output rows per core per batch
KV = E // (H // G)  # 128
LAMBDA_INIT = 0.8

F32 = None  # set lazily (mybir import deferred so host prep works standalone)


# ----------------------------------------------------------------- host prep

def _head_perm():
    """Column permutation applied to Wq1/Wk1: per 64-block -> [evens|odds]."""
    p = []
    for blk in range(0, E, D):
        p += [blk + 2 * j for j in range(D // 2)]
        p += [blk + 2 * j + 1 for j in range(D // 2)]
    return np.array(p, dtype=np.int64)


def _q2_perm():
    """q2: like _head_perm but heads swapped within each 128-col M-tile."""
    base = _head_perm()
    p = np.empty_like(base)
    for m in range(E // 128):
        p[m * 128: m * 128 + 64] = base[m * 128 + 64: m * 128 + 128]
        p[m * 128 + 64: m * 128 + 128] = base[m * 128: m * 128 + 64]
    return p


def _k_perm(swap):
    """kv columns (128 = 2 groups x 64): per group block [evens|odds];
    swap=True puts group1 first (branch-2 layout)."""
    p = []
    groups = (1, 0) if swap else (0, 1)
    for g in groups:
        blk = g * D
        p += [blk + 2 * j for j in range(D // 2)]
        p += [blk + 2 * j + 1 for j in range(D // 2)]
    return np.array(p, dtype=np.int64)


def _tile_w(w, kdim, mdim):
    """(kdim*128, mdim*TS) -> (kdim, mdim, 128, TS) contiguous tiles."""
    ts = w.shape[1] // mdim
    return np.ascontiguousarray(
        w.reshape(kdim, 128, mdim, ts).transpose(0, 2, 1, 3))


def _trig_tables(core):
    pos = (256 * core + np.arange(SEQ, dtype=np.float64))  # global positions
    div = np.exp(np.arange(0, D, 2, dtype=np.float64) * (-np.log(10000.0) / D))
    ang = pos[None, :] * div[:, None]          # (32, SEQ)
    c32 = np.cos(ang).astype(np.float32)
    s32 = np.sin(ang).astype(np.float32)
    tc = np.tile(c32, (4, 1))                   # (128, SEQ)
    # sign-folded sin: rows [0:32]=+sin (qe*sin for the odd half),
    # [32:64]=-sin (-qo*sin for the even half), repeating per 64-block.
    tsn = np.tile(np.concatenate([s32, -s32], axis=0), (2, 1))
    return np.ascontiguousarray(tc), np.ascontiguousarray(tsn)


def _p32():
    """[128,128] permutation: swaps 32-halves within each 64-block.
    Used as matmul lhsT: out = P.T @ u with P[k, m] = 1 iff k = swap(m)."""
    p = np.zeros((128, 128), np.float32)
    for m in range(128):
        k = m + 32 if (m % 64) < 32 else m - 32
        p[k, m] = 1.0
    return p


# ------------------------------------------------------------ device program

_PROGRAM_CACHE = {}


def _build_program():
    import concourse.bass as bass
    import concourse.mybir as mybir
    import concourse.tile as tile
    from concourse.masks import make_identity
    from concourse.tile_rust import add_dep_helper

    def order_group(insts):
        """PE-order a bank-packed accumulation group: first (start=True)
        before everything, last (stop=True) after everything. sync=False —
        same-engine ordering only."""
        first, last = insts[0], insts[-1]
        for i in insts[1:]:
            add_dep_helper(i.ins, first.ins, sync=False,
                           reason="psum group start first")
        for i in insts[:-1]:
            add_dep_helper(last.ins, i.ins, sync=False,
                           reason="psum group stop last")

    f32 = mybir.dt.float32
    f32r = mybir.dt.float32r
    bf16 = mybir.dt.bfloat16
    ALU = mybir.AluOpType
    ACTF = mybir.ActivationFunctionType

    nc = bass.Bass()

    xt_d = nc.dram_tensor("xt", [B, 8, 128, SEQ], bf16, kind="ExternalInput")
    # wq pre-chunked on host: [chunk=(mat,mt//2), 128p, (mt%2, kt, 128)]
    wq_d = nc.dram_tensor("wq", [8, 128, 2048], bf16, kind="ExternalInput")
    wk_d = nc.dram_tensor("wk", [2, 8, 128, 128], bf16, kind="ExternalInput")
    wv_d = nc.dram_tensor("wv", [8, 128, 128], bf16, kind="ExternalInput")
    wo_d = nc.dram_tensor("wo", [8, 2, 128, 512], bf16, kind="ExternalInput")
    tc_d = nc.dram_tensor("tct", [128, SEQ], f32, kind="ExternalInput")
    ts_d = nc.dram_tensor("tst", [128, SEQ], f32, kind="ExternalInput")
    lam_d = nc.dram_tensor("lamv", [128, 2], f32, kind="ExternalInput")
    bout_d = nc.dram_tensor("boutv", [1, E], f32r, kind="ExternalInput")
    p32_d = nc.dram_tensor("p32", [128, 128], bf16, kind="ExternalInput")
    ones_d = nc.dram_tensor("onesv", [1, 128], f32r, kind="ExternalInput")
    y_d = nc.dram_tensor("y", [B, QROWS, E], f32, kind="ExternalOutput")

    def split_matmul_waits():
        """This walrus build allows only ONE sync-wait per engine
        instruction; peel extra waits onto engine-matched no-ops placed
        just before the instruction."""
        for bb in nc.m.functions[0].blocks:
            il = bb.instructions
            new_list = []
            changed = False
            for i in il:
                si = getattr(i, "sync_info", None)
                if si is not None and len(si.on_wait) > 1:
                    waits = list(si.on_wait)
                    for j, w in enumerate(waits[1:]):
                        nop = mybir.InstNoOp(
                            name=f"{i.name}-wnop{j}", engine=i.engine, ins=[],
                            outs=[],
                            sync_info=mybir.SyncInfo(on_wait=[w],
                                                     on_update=[]))
                        nc.inst_map[nop.name] = nop
                        new_list.append(nop)
                    i.sync_info = mybir.SyncInfo(
                        on_wait=[waits[0]], on_update=list(si.on_update))
                    changed = True
                new_list.append(i)
            if changed:
                il[:] = new_list

    with tile.TileContext(nc) as tc:
        with tc.tile_pool(name="const", bufs=1) as constp, \
             tc.tile_pool(name="xt", bufs=1) as xtp, \
             tc.tile_pool(name="rot", bufs=1) as rotp, \
             tc.tile_pool(name="wres", bufs=1) as wresp, \
             tc.tile_pool(name="ru", bufs=3) as rup, \
             tc.tile_pool(name="vext", bufs=1) as vxp, \
             tc.tile_pool(name="att", bufs=4) as attp, \
             tc.tile_pool(name="small", bufs=4) as smp, \
             tc.tile_pool(name="atile", bufs=3) as atp, \
             tc.tile_pool(name="psA", bufs=2, space="PSUM") as psA, \
             tc.tile_pool(name="psSC", bufs=3, space="PSUM") as psSC, \
             tc.tile_pool(name="psPV", bufs=2, space="PSUM") as psPV, \
             tc.tile_pool(name="psTR", bufs=1, space="PSUM") as psTR:

            # x^T + weights: one HW queue (sync), strict consumption order
            xts = {}
            xstrips = []
            for b in range(B):
                strip = xtp.tile([128, 8 * SEQ], bf16, tag=f"xt{b}")
                xstrips.append(strip)
                for kt in range(8):
                    xts[b, kt] = strip[:, kt * SEQ:(kt + 1) * SEQ]

            def dma_x(b):
                nc.sync.dma_start(
                    out=xstrips[b].rearrange("p (k s) -> p k s", k=8),
                    in_=xt_d[b, :, :, :].rearrange("k p s -> p k s"))

            wqr = wresp.tile([128, 16384], bf16, tag="wqr")

            def dma_wq(chunk):
                nc.sync.dma_start(
                    out=wqr[:, chunk * 2048:(chunk + 1) * 2048],
                    in_=wq_d[chunk, :, :])

            def wq_sb(mat, mt, kt):
                off = (mat * 4 + mt // 2) * 2048 + (mt % 2) * 1024 + kt * 128
                return wqr[:, off:off + 128]

            dma_x(0)
            dma_wq(0)
            dma_wq(1)
            tc_sb = constp.tile([128, SEQ], f32, tag="tcs")
            ts_sb = constp.tile([128, SEQ], f32, tag="tss")
            p32_sb = constp.tile([128, 128], bf16, tag="p32s")
            nc.sync.dma_start(out=tc_sb, in_=tc_d[:, :])
            nc.sync.dma_start(out=ts_sb, in_=ts_d[:, :])
            nc.sync.dma_start(out=p32_sb, in_=p32_d[:, :])
            for chunk in range(2, 8):
                dma_wq(chunk)
            dma_x(1)

            wkr = wresp.tile([128, 2048], bf16, tag="wkr")
            nc.sync.dma_start(
                out=wkr.rearrange("p (m k s) -> p m k s", m=2, k=8),
                in_=wk_d[:, :, :, :].rearrange("m k p s -> p m k s"))
            wvr = wresp.tile([128, 1024], bf16, tag="wvr")
            nc.sync.dma_start(
                out=wvr.rearrange("p (k s) -> p k s", k=8),
                in_=wv_d[:, :, :].rearrange("k p s -> p k s"))
            lam_sb = constp.tile([128, 2], f32, tag="lams")
            bout_sb = constp.tile([1, E], f32r, tag="bouts")
            ones1 = constp.tile([1, 128], f32r, tag="ones1")
            nc.sync.dma_start(out=lam_sb, in_=lam_d[:, :])
            nc.sync.dma_start(out=bout_sb, in_=bout_d[:, :])
            nc.sync.dma_start(out=ones1, in_=ones_d[:, :])
            wor = wresp.tile([128, 8192], bf16, tag="wor")
            nc.sync.dma_start(
                out=wor.rearrange("p (k n s) -> p k n s", k=8, n=2),
                in_=wo_d[:, :, :, :].rearrange("k n p s -> p k n s"))
            wo_sb = {}
            for kt in range(8):
                for nh in range(2):
                    off = kt * 1024 + nh * 512
                    wo_sb[kt, nh] = wor[:, off:off + 512]

            identf = constp.tile([128, 128], f32, tag="identf")
            make_identity(nc, identf)
            identb = constp.tile([128, 128], bf16, tag="identb")
            nc.vector.tensor_copy(identb, identf)

            def rope(psum_in, rot_out):
                # rot = psum*TC + P32 @ (psum*TS_signed)
                # (the PE matmul does the cross-partition 32-half swap that
                # DVE cannot: walrus requires same start partition on all
                # InstTensorTensor operands)
                t = rup.tile([128, SEQ], f32, tag="ropet")
                u = rup.tile([128, SEQ], bf16, tag="ropeu")
                nc.vector.tensor_mul(t, psum_in, tc_sb)
                nc.vector.tensor_mul(u, psum_in, ts_sb)
                usw = psSC.tile([128, 512], f32, tag="sc")
                nc.tensor.matmul(usw[:, 0:SEQ], p32_sb,
                                 u, start=True, stop=True)
                nc.vector.tensor_add(rot_out, t, usw[:, 0:SEQ])

            # ---- q projections + rope ----
            qrot = {}
            for b in range(B):
                for mat in range(2):
                    for mt in range(8):
                        ps = psA.tile([128, 512], f32, tag="proj")
                        qp = ps[:, 0:SEQ]
                        for kt in range(8):
                            nc.tensor.matmul(
                                qp, wq_sb(mat, mt, kt), xts[b, kt],
                                start=(kt == 0), stop=(kt == 7))
                        rot = rotp.tile([128, SEQ], bf16, tag=f"q{mat}_{b}_{mt}")
                        rope(qp, rot)
                        qrot[mat, b, mt] = rot

            # ---- k projections + rope ----
            krot = {}
            for mat in range(2):
                for b in range(B):
                    ps = psA.tile([128, 512], f32, tag="proj")
                    kp = ps[:, 0:SEQ]
                    for kt in range(8):
                        nc.tensor.matmul(
                            kp, wkr[:, mat * 1024 + kt * 128:
                                    mat * 1024 + kt * 128 + 128],
                            xts[b, kt],
                            start=(kt == 0), stop=(kt == 7))
                    rot = rotp.tile([128, SEQ], bf16, tag=f"k{mat}_{b}")
                    rope(kp, rot)
                    krot[mat, b] = rot

            # ---- v projections (transposed, full-rate) + v_ext tiles ----
            vext = {}
            for b in range(B):
                # v^T [kv=128, SEQ] at full bf16 rate (N=384)
                ps = psA.tile([128, 512], f32, tag="proj")
                vtp = ps[:, 0:SEQ]
                for kt in range(8):
                    nc.tensor.matmul(
                        vtp, wvr[:, kt * 128:(kt + 1) * 128],
                        xts[b, kt],
                        start=(kt == 0), stop=(kt == 7))
                vt_sb = rup.tile([128, SEQ], bf16, tag="vtsb")
                nc.vector.tensor_copy(vt_sb, vtp)
                # PE-transpose each 128-seq chunk back to [seq, kv]
                for st in range(3):
                    vtr = psTR.tile([128, 256], bf16, tag="trp")
                    nc.tensor.matmul(vtr[:, 0:128],
                                     vt_sb[:, st * 128:(st + 1) * 128],
                                     identb, is_transpose=True)
                    for g in range(2):
                        ve = vxp.tile([128, 65], bf16,
                                      tag=f"ve_{b}_{st}_{g}")
                        nc.vector.tensor_copy(ve[:, 0:64],
                                              vtr[:, g * 64:(g + 1) * 64])
                        nc.gpsimd.tensor_copy(ve[:, 64:65],
                                              lam_sb[:, 0:1])
                        vext[b, st, g] = ve

            # broadcast bias tile [128, 1024] via two K=1 matmuls (one-time)
            bias_sb = constp.tile([128, E], f32, tag="biasbc")
            for nh in range(2):
                bps = psA.tile([128, 512], f32, tag="proj")
                nc.tensor.matmul(bps, ones1,
                                 bout_sb[:, nh * 512:(nh + 1) * 512],
                                 start=True, stop=True)
                nc.vector.tensor_copy(bias_sb[:, nh * 512:(nh + 1) * 512],
                                      bps)

            # ---- attention + output projection ----
            for b in range(B):
                for w in range(NW):
                    at_sb = {}
                    for h in range(H):
                        g = h & 1
                        mt = h >> 1
                        base1 = 64 * g          # branch-1 partition base
                        base2 = 64 - base1      # branch-2 partition base
                        e_sb = []
                        for br, qb in ((0, base1), (1, base2)):
                            st_ps = psSC.tile([128, 512], f32, tag="sc")
                            sc_mms = []
                            for kts in range(2):
                                sc_mms.append(nc.tensor.matmul(
                                    st_ps[:, kts * 256:(kts + 1) * 256],
                                    krot[br, b][qb:qb + 64,
                                                w * 128 + kts * 128:
                                                w * 128 + kts * 128 + 128]
                                    ,
                                    qrot[br, b, mt][qb:qb + 64,
                                                    w * 128:w * 128 + 256]
                                    ,
                                    start=(kts == 0), stop=(kts == 1)))
                            order_group(sc_mms)
                            e = attp.tile([128, 512], bf16, tag=f"e{br}")
                            nc.scalar.activation(e, st_ps, ACTF.Exp,
                                                 scale=0.125)
                            e_sb.append(e)

                        pv = psPV.tile([128, 260], f32, tag="pv")
                        pv_mms = []
                        first = True
                        for kts in range(2):
                            for br in range(2):
                                for qt in range(2):
                                    col = (br * 2 + qt) * 65
                                    pv_mms.append(nc.tensor.matmul(
                                        pv[:, col:col + 65],
                                        e_sb[br][:, kts * 256 + qt * 128:
                                                 kts * 256 + qt * 128 + 128]
                                        ,
                                        vext[b, w + kts, g],
                                        start=first,
                                        stop=(kts == 1 and br == 1 and qt == 1)))
                                    first = False
                        order_group(pv_mms)

                        r_sb = smp.tile([128, 6], f32, tag="recip")
                        nc.vector.reciprocal(r_sb[:, 0:4], pv[:, 64:260:65])
                        # fold lambda into the branch-2 reciprocals (f32)
                        nc.vector.tensor_scalar_mul(
                            r_sb[:, 4:6], r_sb[:, 2:4], lam_sb[:, 1:2])

                        if g == 0:
                            # [128 q, 128] pair tile: h-even dims in cols
                            # 0:64, h-odd in 64:128; transposed in one shot.
                            pair_sb = [smp.tile([128, 128], bf16,
                                                tag=f"pair{qt}",
                                                name=f"pair{qt}")
                                       for qt in range(2)]
                        for qt in range(2):
                            t2 = smp.tile([128, 64], f32, tag="t2")
                            nc.scalar.activation(
                                t2, pv[:, 130 + qt * 65:130 + qt * 65 + 64],
                                ACTF.Copy, scale=r_sb[:, 4 + qt:5 + qt])
                            nc.vector.scalar_tensor_tensor(
                                out=pair_sb[qt][:, g * 64:(g + 1) * 64],
                                in0=pv[:, qt * 65:qt * 65 + 64],
                                scalar=r_sb[:, qt:qt + 1], in1=t2,
                                op0=ALU.mult, op1=ALU.subtract)
                        if g == 1:
                            tr_ps = psTR.tile([128, 256], bf16, tag="trp")
                            for qt in range(2):
                                nc.tensor.transpose(
                                    tr_ps[:, qt * 128:(qt + 1) * 128],
                                    pair_sb[qt], identb)
                                at = atp.tile([128, 128], bf16,
                                              tag=f"at{mt}_{qt}")
                                nc.vector.tensor_copy(
                                    at, tr_ps[:, qt * 128:(qt + 1) * 128])
                                at_sb[mt, qt] = at

                    for qt in range(2):
                        for nh in range(2):
                            y_ps = psA.tile([128, 512], f32, tag="proj")
                            y_mms = []
                            for kt in range(8):
                                y_mms.append(nc.tensor.matmul(
                                    y_ps, at_sb[kt, qt],
                                    wo_sb[kt, nh],
                                    start=(kt == 0), stop=(kt == 7)))
                            order_group(y_mms)
                            y_sb = smp.tile([128, 512], f32, tag="ysb")
                            nc.vector.tensor_add(
                                y_sb, y_ps,
                                bias_sb[:, nh * 512:(nh + 1) * 512])
                            nc.gpsimd.dma_start(
                                out=y_d[b, (w * 2 + qt) * 128:
                                        (w * 2 + qt) * 128 + 128,
                                        nh * 512:(nh + 1) * 512],
                                in_=y_sb)
    split_matmul_waits()
    return nc


def get_program():
    if "nc" not in _PROGRAM_CACHE:
        _PROGRAM_CACHE["nc"] = _build_program()
    return _PROGRAM_CACHE["nc"]


# ------------------------------------------------------------------ host API

def make_in_maps(x, Wq1, Wq2, Wk1, Wk2, Wv, Wout, bout, lq1, lk1, lq2, lk2):
    import ml_dtypes
    bf16 = ml_dtypes.bfloat16

    x = np.asarray(x, dtype=np.float32)
    lam = float(np.clip(
        np.exp(np.asarray(lq1, np.float64) @ np.asarray(lk1, np.float64))
        - np.exp(np.asarray(lq2, np.float64) @ np.asarray(lk2, np.float64))
        + LAMBDA_INIT, 0.1, 0.9))

    qp1, qp2 = _head_perm(), _q2_perm()
    kp1, kp2 = _k_perm(False), _k_perm(True)

    wq_t = np.stack([
        _tile_w(np.asarray(Wq1, np.float32)[:, qp1], 8, 8),
        _tile_w(np.asarray(Wq2, np.float32)[:, qp2], 8, 8),
    ])  # (mat, kt, mt, 128, 128)
    # chunk layout for single contiguous DMAs:
    # (mat, mt//2, p, mt%2, kt, s) -> (8, 128, 2048)
    wq = np.ascontiguousarray(
        wq_t.reshape(2, 8, 4, 2, 128, 128)
        .transpose(0, 2, 4, 3, 1, 5).reshape(8, 128, 2048)).astype(bf16)
    wk = np.stack([
        _tile_w(np.asarray(Wk1, np.float32)[:, kp1], 8, 1)[:, 0],
        _tile_w(np.asarray(Wk2, np.float32)[:, kp2], 8, 1)[:, 0],
    ]).astype(bf16)  # (2, 8, 128, 128)
    wv = _tile_w(np.asarray(Wv, np.float32), 8, 1)[:, 0].astype(bf16)
    wo = _tile_w(np.asarray(Wout, np.float32), 8, 2).astype(bf16)
    boutv = np.asarray(bout, np.float32).reshape(1, E)

    lamv = np.zeros((128, 2), np.float32)
    lamv[:, 0] = 1.0     # exact ones column for the shared denominator
    lamv[:, 1] = lam     # lambda, applied in f32 on DVE

    # x^T, tiled: (B, 8, 128, SEQ) per core
    xT = np.ascontiguousarray(x.transpose(0, 2, 1))  # (B, E, L)

    in_maps = []
    for c in range(NCORES):
        s0 = 256 * c
        xt = np.ascontiguousarray(
            xT[:, :, s0:s0 + SEQ].reshape(B, 8, 128, SEQ)).astype(bf16)
        tct, tst = _trig_tables(c)
        in_maps.append({
            "xt": xt, "wq": wq, "wk": wk, "wv": wv, "wo": wo,
            "tct": tct, "tst": tst, "lamv": lamv, "boutv": boutv,
            "p32": _p32().astype(bf16),
            "onesv": np.ones((1, 128), np.float32),
        })
    return in_maps


def kernel(**inputs) -> np.ndarray:
    from concourse.bass_utils import run_bass_kernel_spmd

    in_maps = make_in_maps(**inputs)
    nc = get_program()
    res = run_bass_kernel_spmd(nc, in_maps, core_ids=list(range(NCORES)))
    out = np.empty((B, L, E), dtype=np.float32)
    for c in range(NCORES):
        out[:, 512 * c:512 * (c + 1), :] = res.results[c]["y"]
    return out
